# revision 5
# baseline (speedup 1.0000x reference)
"""Trainium2 Bass kernel for nn_CenterLossN (center-loss style reduction).

Math (per batch n, class c; H=W=384, C=11, N=32):
    res[n,c]   = x[n,c]^2 + centers[n,c]^2 - 2 * x[n,c] @ centers[n,c]
    out[n,h,w] = max_c softmax_c(res)[n,c,h,w] = 1 / sum_c exp(res_c - max_c res_c)
    loss       = sum(clip(out * labels, 1e-12, 1e12)) / (N*H*W)

Device strategy (data-parallel over N across 8 cores, 4 batches/core):
  Host ships, per (n,c) plane, three bf16 [384,384] arrays:
    xt2 = (-2*x)^T          -> matmul lhsT (PSUM gets -2*x@c directly)
    cc  = centers           -> matmul rhs
    ee  = x^2 + centers^2   -> injected into the same PSUM accumulation
                               via an identity-matmul (PSUM += I^T @ ee)
  so PSUM ends up holding s = res in fp32 with zero vector-engine work.
  Then per 128-row chunk: ACT copies each class plane PSUM->SBUF bf16,
  DVE running-max over 11 classes, DVE subtract, one batched ACT exp over
  all 11 planes, DVE add-chain, t = exp(-ln(acc)) on ACT, and a fused
  multiply(label)+reduce into per-partition partials.
  clip: only label==0 hits the 1e-12 floor (1/sum >= 1/11 and <= 1);
  host adds 1e-12 * count(labels==0) exactly.
"""

import numpy as np
import ml_dtypes

N, C, H, W = 32, 11, 384, 384
N_CORES = 8
N_LOC = N // N_CORES          # 4 batches per core
PAIRS = N_LOC * C             # 44 (n,c) planes per core
MC = H // 128                 # 3 row-chunks
KC = W // 128                 # 3 contraction chunks

_BF16 = ml_dtypes.bfloat16
_COMPILED = None


def _build(n_loc=N_LOC, n_cls=C):
    from contextlib import ExitStack
    import concourse.bacc as bacc
    import concourse.tile as tile
    from concourse import mybir

    bf16 = mybir.dt.bfloat16
    f32 = mybir.dt.float32
    AF = mybir.ActivationFunctionType

    nc = bacc.Bacc("TRN2", target_bir_lowering=False, debug=False)

    pairs = n_loc * n_cls
    xt2_d = nc.dram_tensor("xt2", [pairs, W, H], bf16, kind="ExternalInput")
    cc_d = nc.dram_tensor("cc", [pairs, W, H], bf16, kind="ExternalInput")
    ee_d = nc.dram_tensor("ee", [pairs, H, W], bf16, kind="ExternalInput")
    lab_d = nc.dram_tensor("lab", [n_loc, H, W], bf16, kind="ExternalInput")
    id_d = nc.dram_tensor("ident", [128, 128], bf16, kind="ExternalInput")
    out_d = nc.dram_tensor("out", [128, 1], f32, kind="ExternalOutput")

    with ExitStack() as ctx:
        tc = ctx.enter_context(tile.TileContext(nc))
        loads = ctx.enter_context(tc.tile_pool(name="loads", bufs=3))
        spool = ctx.enter_context(tc.tile_pool(name="spool", bufs=6))
        dpool = ctx.enter_context(tc.tile_pool(name="dpool", bufs=3))
        epool = ctx.enter_context(tc.tile_pool(name="epool", bufs=3))
        small = ctx.enter_context(tc.tile_pool(name="small", bufs=4))
        singles = ctx.enter_context(tc.tile_pool(name="singles", bufs=1))
        psum = ctx.enter_context(tc.tile_pool(name="psum", bufs=8, space="PSUM"))

        ident_t = singles.tile([128, 128], bf16)
        nc.sync.dma_start(ident_t[:], id_d[:, :])
        partial = singles.tile([128, n_loc * MC], f32)

        for n in range(n_loc):
            s_tiles = [
                spool.tile([128, n_cls, W], bf16, tag="S", name=f"S_{n}_{mc}")
                for mc in range(MC)
            ]
            for c in range(n_cls):
                i = n * n_cls + c
                xt2_t = loads.tile([128, KC, H], bf16, tag="xt2")
                nc.sync.dma_start(
                    xt2_t[:], xt2_d[i].rearrange("(kc p) h -> p kc h", p=128)
                )
                cc_t = loads.tile([128, KC, W], bf16, tag="cc")
                nc.sync.dma_start(
                    cc_t[:], cc_d[i].rearrange("(kc p) w -> p kc w", p=128)
                )
                ee_t = loads.tile([128, MC, W], bf16, tag="ee")
                nc.sync.dma_start(
                    ee_t[:], ee_d[i].rearrange("(mc p) w -> p mc w", p=128)
                )
                for mc in range(MC):
                    ps = psum.tile([128, W], f32, tag="ps")
                    for kc in range(KC):
                        nc.tensor.matmul(
                            ps[:],
                            xt2_t[:, kc, mc * 128 : (mc + 1) * 128],
                            cc_t[:, kc, :],
                            start=(kc == 0),
                            stop=False,
                        )
                    nc.tensor.matmul(
                        ps[:], ident_t[:], ee_t[:, mc, :], start=False, stop=True
                    )
                    # s (=res) fp32 -> bf16, PSUM -> SBUF on the scalar engine
                    nc.scalar.copy(s_tiles[mc][:, c, :], ps[:])

            for mc in range(MC):
                S = s_tiles[mc]
                m = small.tile([128, W], bf16, tag="m")
                nc.vector.tensor_max(m[:], S[:, 0, :], S[:, 1, :])
                for c in range(2, n_cls):
                    nc.vector.tensor_max(m[:], m[:], S[:, c, :])
                d = dpool.tile([128, n_cls, W], bf16, tag="D")
                for c in range(n_cls):
                    nc.vector.tensor_sub(d[:, c, :], S[:, c, :], m[:])
                e = epool.tile([128, n_cls, W], bf16, tag="E")
                nc.scalar.activation(e[:], d[:], AF.Exp)
                acc = small.tile([128, W], bf16, tag="acc")
                nc.vector.tensor_add(acc[:], e[:, 0, :], e[:, 1, :])
                for c in range(2, n_cls):
                    nc.vector.tensor_add(acc[:], acc[:], e[:, c, :])
                # t = 1/acc (exact iterative divide on DVE; Ln/TTR/custom-DVE
                # are not available on this deployment)
                t = small.tile([128, W], f32, tag="t")
                nc.vector.reciprocal(t[:], acc[:])
                labt = loads.tile([128, W], bf16, tag="lab")
                nc.sync.dma_start(labt[:], lab_d[n, mc * 128 : (mc + 1) * 128, :])
                w_t = small.tile([128, W], f32, tag="w")
                nc.vector.tensor_mul(w_t[:], t[:], labt[:])
                slot = n * MC + mc
                nc.vector.tensor_reduce(
                    partial[:, slot : slot + 1],
                    w_t[:],
                    axis=mybir.AxisListType.X,
                    op=mybir.AluOpType.add,
                )

        pf = singles.tile([128, 1], f32)
        nc.vector.tensor_reduce(
            pf[:], partial[:], axis=mybir.AxisListType.X, op=mybir.AluOpType.add
        )
        nc.sync.dma_start(out_d[:, :], pf[:])

    nc.compile()
    return nc


def _get_compiled():
    global _COMPILED
    if _COMPILED is None:
        _COMPILED = _build()
    return _COMPILED


def _host_prep(x, centers, labels):
    x = np.asarray(x, dtype=np.float32)
    centers = np.asarray(centers, dtype=np.float32)
    labels_np = np.asarray(labels)

    n_zero = int((labels_np == 0).sum())

    xt2 = np.ascontiguousarray(
        np.transpose(-2.0 * x, (0, 1, 3, 2))
    ).astype(_BF16)                       # (N, C, W, H)
    cc = centers.astype(_BF16)            # (N, C, H, W)
    ee = (x * x + centers * centers).astype(_BF16)
    lab = labels_np.astype(np.float32).astype(_BF16)  # (N, H, W), values 0..10 exact
    ident = np.eye(128, dtype=_BF16)

    in_maps = []
    for core in range(N_CORES):
        sl = slice(core * N_LOC, (core + 1) * N_LOC)
        in_maps.append(
            {
                "xt2": np.ascontiguousarray(xt2[sl]).reshape(PAIRS, W, H),
                "cc": np.ascontiguousarray(cc[sl]).reshape(PAIRS, H, W),
                "ee": np.ascontiguousarray(ee[sl]).reshape(PAIRS, H, W),
                "lab": np.ascontiguousarray(lab[sl]),
                "ident": ident,
            }
        )
    return in_maps, n_zero


def kernel(x, centers, labels, _trace=False, _trace_kwargs=None):
    from concourse import bass_utils

    nc = _get_compiled()
    in_maps, n_zero = _host_prep(x, centers, labels)

    kwargs = {}
    if _trace:
        kwargs = dict(trace=True, **(_trace_kwargs or {}))
    res = bass_utils.run_bass_kernel_spmd(
        nc, in_maps, core_ids=list(range(N_CORES)), **kwargs
    )

    total = 0.0
    for core in range(N_CORES):
        total += float(res.results[core]["out"].astype(np.float64).sum())
    loss = (total + 1e-12 * n_zero) / float(N * H * W)
    out = np.float32(loss)
    if _trace:
        return out, res
    return out
